# revision 40
# baseline (speedup 1.0000x reference)
"""Trainium2 Bass kernel for nn_GATModel (GATv2 on a bidirectional chain graph).

Key algebraic facts exploited (derived from the reference):
  * The reference's conv loop feeds x0 into EVERY layer, so only the LAST
    GATv2 layer (index L-1) affects the output.
  * x0 = x @ W_exp + b_exp + pe  never needs materializing:
        xl = x0 @ Wl + bl = x @ (W_exp@Wl) + [(b_exp+pe[n])@Wl + bl]
    i.e. a [64,256] matmul plus a per-node (n) bias.
  * The graph is a chain + self loops, so message passing is a 3-tap stencil
    (left / self / right) with a masked 3-way softmax per node.
  * a . leaky_relu(z) = 0.6*(a . z) + 0.4*(a . |z|)   (slope 0.2)
    and with ahat=|a| folded into the weight columns (positively homogeneous),
    a_h*|z_h| = sign(a_h)*|ztilde_h|.  So the nonlinear part is a signed sum
    of |ztilde| and the linear part is two per-node scalars (p, q).

Device pipeline per 500-row chunk (col-major z: [h-part, row-free]):
  z_sigma in PSUM via matmul accumulation: a rank-64 factorized per-node
  bias matmul (start=True; pe's numerical rank is ~40 so this is exact)
  + a K=128 concat data matmul ([x(j+-1); x(j)] @ [Wl~; Wr~] against an
  x^T tile holding the shifted copy on partitions 0:64), interleaved per
  stencil so next-stencil LDWEIGHTS hides under the running matmul;
  -> |z~| crossing PSUM->SBUF into BF16 tiles, split between VectorE
  (tensor_scalar abs_max(z,0)) and ScalarE (Abs) so both engines run
  concurrently;
  -> t_sigma = sum_h sign(a_h)|z~| via M=1 bf16 PE matmuls into one PSUM
  bank (partitions 0/32/64; p,q,y rows at 96..101 via a concurrent
  col-tiled matmul at tile_position (64,96)).  The three stencils' M=1
  matmuls sit at distinct col groups so they overlap in the array.
  The t-phase of chunk c is issued after the z matmuls of chunk c+1
  (software pipeline), hiding the evacuation latency.
Host finishes: logits = 0.6(p+q) + 0.4 t, masks, 3-way softmax, alpha-
weighted message pooling, final fc - O(B*N) work; all O(B*N*H) is on HW.

Note: the first execution of a freshly compiled NEFF intermittently hits
NRT_EXEC_UNIT_UNRECOVERABLE on this axon stack; kernel() retries.
"""

import os
import sys

sys.path.insert(0, "/opt/trn_rl_repo")

from contextlib import ExitStack  # noqa: E402

import ml_dtypes  # noqa: E402
import numpy as np  # noqa: E402

import concourse.bass as bass  # noqa: E402
import concourse.tile as tile  # noqa: E402
from concourse import bacc, mybir  # noqa: E402
from concourse.bass_utils import run_bass_kernel_spmd  # noqa: E402

BF16 = mybir.dt.bfloat16
F32 = mybir.dt.float32
FP8 = mybir.dt.float8e4
NPBF16 = ml_dtypes.bfloat16
NPFP8 = ml_dtypes.float8_e4m3

B, N, IN, H, L, C = 2048, 100, 64, 256, 3, 3
NEG = 0.2
NCORES = 8
BC = B // NCORES            # 256 graphs per core
ROWS = BC * N               # 25600 rows per core
CHF = 512                   # rows per chunk (25600 = 50 * 512 exactly)
NCH = ROWS // CHF           # 50 chunks
BMW = 640                   # bias-basis pattern width (period 100, offsets<=96)
CHUNKS = [(i * CHF, CHF) for i in range(NCH)]
XP = 25616                  # padded x8 block width (stride must be 16-aligned)
XH = 12816                  # x8 half-tile width (2*XH must fit 16-bit AP step)
XB = 12800                  # first row covered by the second half-tile
FP8SC = 64.0                # fp8 weight scale (power of two, exact to undo)

LAST_RESULTS = None  # set by kernel() for test harness inspection


def _make_pe_np(n, d):
    pos = np.arange(n, dtype=np.float32)[:, None]
    div = np.exp(
        np.arange(0, d, 2, dtype=np.float32) * (-np.log(np.float32(10000.0)) / d)
    )
    pe = np.zeros((n, d), dtype=np.float32)
    pe[:, 0::2] = np.sin(pos * div)
    pe[:, 1::2] = np.cos(pos * div)
    return pe


def _install_profile_shim():
    """Best-effort: register the NTFF profile hook this container's antenv
    lacks, so BASS_TRACE=1 produces exec_time_ns instead of crashing."""
    try:
        import types

        if "antenv.axon_hooks" in sys.modules:
            return
        if "/root/.axon_site" not in sys.path:
            sys.path.insert(0, "/root/.axon_site")
        from trn_agent_boot.trn_boot import _ntff_profile_via_ctypes

        hook = _ntff_profile_via_ctypes("/opt/axon/libaxon_pjrt.so")
        mod = types.ModuleType("antenv.axon_hooks")
        mod.get_axon_ntff_profile_hook = lambda: hook
        mod.set_axon_ntff_profile_hook = lambda h: None
        sys.modules["antenv.axon_hooks"] = mod
        import antenv

        antenv.axon_hooks = mod
        import concourse.bass_utils as _bu

        _bu.upload_artifacts = lambda d: f"local://{d}"
    except Exception:
        pass


_install_profile_shim()

_PROG_CACHE = None


def _build_program():
    """Build the (shape-only) Bass program once; weights arrive via in_maps."""
    nc = bacc.Bacc(
        "TRN2",
        target_bir_lowering=False,
        debug=False,
        enable_asserts=False,
        num_devices=NCORES,
    )

    d_in = {}

    def din(name, shape, dt):
        d_in[name] = nc.dram_tensor(name, list(shape), dt, kind="ExternalInput").ap()
        return d_in[name]

    CONSTS = din("CONSTS", (128, 2), BF16)   # CO sign columns only
    # fp8 DoubleRow operands: X8 K-blocks ordered [basis_s|basis_r|x|basis_l]
    # so every stencil's (even,odd) K pair is a stride-XH or stride-2XH view;
    # C8 = 6 stationaries (l/r/s x h-block) as matching weight-block pairs
    X8 = din("X8", (128, 2, 4, XH), FP8)
    C8D = din("C8D", (128, 12, 128), FP8)
    outsT_dram = nc.dram_tensor("outsT", [3, ROWS], F32, kind="ExternalOutput").ap()

    ZW = 1024                      # per-stencil psum tile width (2 banks)

    with tile.TileContext(nc) as tc, ExitStack() as ctx:
        cpool = ctx.enter_context(tc.tile_pool(name="consts", bufs=1))
        x3pool = ctx.enter_context(tc.tile_pool(name="x3", bufs=1))
        zpool = ctx.enter_context(
            tc.tile_pool(name="z", bufs=1, space=bass.MemorySpace.PSUM)
        )
        tbpool = ctx.enter_context(
            tc.tile_pool(name="tb", bufs=1, space=bass.MemorySpace.PSUM)
        )
        wpool = ctx.enter_context(tc.tile_pool(name="w", bufs=3))
        spool = ctx.enter_context(tc.tile_pool(name="stage", bufs=2))

        # psum: 3 z tiles of [128, 1024] f32 (= 2 banks each, bank aligned)
        # + 1 double-wide tb tile [128, 1024] (2 banks, one per chunk parity)
        # = exactly 8 banks
        zt = {}
        for s in ("l", "r", "s"):
            zt[s] = zpool.tile([128, ZW], F32, tag=f"z{s}", name=f"z{s}")
        tbt = tbpool.tile([128, ZW], F32, tag="tb", name="tb")

        def zslice(s, b, F):
            return zt[s][:, b * 512 : b * 512 + F]

        CT = cpool.tile([128, 2], BF16, tag="c_all", name="c_all")
        nc.sync.dma_start(CT[:], CONSTS[:])
        c8 = cpool.tile([128, 12, 128], FP8, tag="c8", name="c8")
        nc.gpsimd.dma_start(c8[:], C8D[:])
        CO = CT[:, 0:2]

        x8a = x3pool.tile([128, 4, XH], FP8, tag="x8a", name="x8a")
        x8b = x3pool.tile([128, 4, XH], FP8, tag="x8b", name="x8b")
        # fine-grained pieces in consumption order so no chunk waits on a
        # bulk transfer; alternate DGE engines
        sizes8a = [512, 1024, 1536, 2048, 2048, 2560, 3088]
        assert sum(sizes8a) == XH
        sizes8b = [2048, 2048, 2560, 3072, 3088]
        assert sum(sizes8b) == XH
        x8ops = []
        a = 0
        for sz in sizes8a:
            x8ops.append((x8a, 0, a, a + sz))
            a += sz
        a = 0
        for sz in sizes8b:
            x8ops.append((x8b, 1, a, a + sz))
            a += sz
        for i, (t8, h8, a8, b8) in enumerate(x8ops):
            eng = nc.gpsimd if i % 2 == 0 else nc.sync
            eng.dma_start(t8[:, :, a8:b8], X8[:, h8, :, a8:b8])
        

        # ---- HAM warmup: keep PE busy during the initial x8 DMA wait so the
        # clock gate opens before real work; plain fp8 matmuls on the const
        # tile (garbage values, overwritten by chunk 0's start=True).
        for i in range(8):
            nc.tensor.matmul(zslice(("l", "r", "s")[i % 3], 0, 512),
                             c8[:, 0, :], c8[:, 2 * (i % 4) : 2 * (i % 4) + 4, :],
                             start=True, stop=True)

        prev = None  # (tb_tile, w_all, c0, F, ci)

        DRM = mybir.MatmulPerfMode.DoubleRow

        def emit_zphase(ci, c0, F):
            # one fp8 DoubleRow matmul per (stencil, h-block): virtual K=256
            # = x_cat data (128) + rank-128 per-node bias basis (128).
            # X8 block order [bS|bR|x|bL]; stationary halves match pair order.
            x8t = x8a if c0 < XB else x8b
            lc0 = c0 if c0 < XB else c0 - XB
            for b in (0, 1):
                nc.tensor.matmul(zslice("l", b, F), c8[:, 2 * b : 2 * b + 2, :],
                                 x8t[:, 2:4, lc0 : lc0 + F],
                                 start=True, stop=True, perf_mode=DRM)
            for b in (0, 1):
                nc.tensor.matmul(zslice("r", b, F), c8[:, 4 + 2 * b : 6 + 2 * b, :],
                                 x8t[:, 1:3, lc0 + 1 : lc0 + 1 + F],
                                 start=True, stop=True, perf_mode=DRM)
            for b in (0, 1):
                nc.tensor.matmul(zslice("s", b, F), c8[:, 8 + 2 * b : 10 + 2 * b, :],
                                 x8t[:, 0:3:2, lc0 : lc0 + F],
                                 start=True, stop=True, perf_mode=DRM)

        def emit_evac(ci, F):
            # relu(z~) PSUM -> SBUF bf16, one [128, 512+F] op per stencil
            # (a.lrelu(z) = 0.2(a.z) + 0.8 sum_h sign(a_h) relu(z~_h));
            # all six w blocks land in ONE tile so the t matvecs stream from
            # a single source.
            w_all = wpool.tile([128, 3 * ZW], BF16, tag="wall", name="wall")
            for si, s in enumerate(("l", "r", "s")):
                dst = w_all[:, si * ZW : si * ZW + 512 + F]
                src_ = zt[s][:, 0 : 512 + F]
                if (si + ci) % 2 == 0:
                    nc.vector.tensor_scalar(dst, src_, 0.0, None,
                                            mybir.AluOpType.max)
                else:
                    nc.scalar.activation(dst, src_,
                                         mybir.ActivationFunctionType.Relu)
            return w_all

        def emit_tphase(ci, w_all, F):
            # strict per-stencil group order (whole-bank has_written clear on
            # start=True); cross-stencil overlap comes from distinct col grps.
            # chunk parity selects the tb bank (cols 0:512 / 512:1024).
            o = 512 * (ci % 2)
            for si, s in enumerate(("l", "r", "s")):
                p0 = 32 * si
                nc.tensor.matmul(tbt[p0 : p0 + 1, o : o + F], CO[:, 0:1],
                                 w_all[:, si * ZW : si * ZW + F],
                                 start=True, stop=False)
                nc.tensor.matmul(tbt[p0 : p0 + 1, o : o + F], CO[:, 1:2],
                                 w_all[:, si * ZW + 512 : si * ZW + 512 + F],
                                 start=False, stop=True)

        def emit_tail(ci, c0, F):
            # one copy + DMA per chunk pair (ci odd covers [ci-1, ci])
            if ci % 2 == 0:
                return
            st = spool.tile([65, ZW], F32, tag="stA")
            if (ci // 2) % 2 == 0:
                nc.vector.tensor_copy(st[0:65, :], tbt[0:65, :])
            else:
                nc.scalar.copy(st[0:65, :], tbt[0:65, :])
            pc0 = c0 - CHF  # start of the even partner chunk
            nc.gpsimd.dma_start(outsT_dram[0:3, pc0 : pc0 + 2 * CHF],
                                st[0:65:32, 0 : 2 * CHF])

        pend = []
        for ci, (c0, F) in enumerate(CHUNKS):
            emit_zphase(ci, c0, F)
            tp = None
            if len(pend) >= 2:
                pw, pc0_, pF, pci = pend.pop(0)
                emit_tphase(pci, pw, pF)
                tp = (pci, pc0_, pF)
            w_all = emit_evac(ci, F)
            if tp is not None:
                emit_tail(*tp)
            pend.append((w_all, c0, F, ci))

        for pw, pc0_, pF, pci in pend:
            emit_tphase(pci, pw, pF)
            emit_tail(pci, pc0_, pF)

    nc.compile()
    return nc


def _get_program():
    global _PROG_CACHE
    if _PROG_CACHE is None:
        _PROG_CACHE = _build_program()
    return _PROG_CACHE


def kernel(x, W_exp, b_exp, W_l, b_l, W_r, b_r, att, bias, W_fc, b_fc):
    global LAST_RESULTS
    x = np.asarray(x, dtype=np.float32)
    W_exp = np.asarray(W_exp, np.float32)
    b_exp = np.asarray(b_exp, np.float32)
    W_l = np.asarray(W_l, np.float32)
    b_l = np.asarray(b_l, np.float32)
    W_r = np.asarray(W_r, np.float32)
    b_r = np.asarray(b_r, np.float32)
    att = np.asarray(att, np.float32)
    bias = np.asarray(bias, np.float32)
    W_fc = np.asarray(W_fc, np.float32)
    b_fc = np.asarray(b_fc, np.float32)

    lw = L - 1  # only the last conv layer matters
    pe = _make_pe_np(N, H)
    a = att[lw]
    s = np.where(a >= 0.0, 1.0, -1.0).astype(np.float32)
    ahat = np.abs(a)

    Wl_full = W_exp @ W_l[lw]                     # [64,256]
    Wr_full = W_exp @ W_r[lw]
    cl = (b_exp + pe) @ W_l[lw] + b_l[lw]         # [100,256]
    cr = (b_exp + pe) @ W_r[lw] + b_r[lw]

    Wtl = Wl_full * ahat[None, :]                 # ahat-folded
    Wtr = Wr_full * ahat[None, :]
    ctl = cl * ahat[None, :]
    ctr = cr * ahat[None, :]

    # stationaries [K,M]: K = concat feature dim, M = h-block columns
    def blk(Wm, b):
        return Wm[:, b * 128 : (b + 1) * 128]

    def bf(arr):
        return np.ascontiguousarray(arr.astype(NPBF16))

    consts = {}
    S_lr_np = [np.concatenate([blk(Wtl, b), blk(Wtr, b)], axis=0) for b in (0, 1)]
    S_rl_np = [np.concatenate([blk(Wtr, b), blk(Wtl, b)], axis=0) for b in (0, 1)]
    Wts = Wtl + Wtr
    # s-stencil x-weights: zeros on the x(i-1) half, Wts block on the x(i) half
    S_s_np = [np.concatenate([np.zeros((64, 128), np.float32), blk(Wts, b)],
                             axis=0) for b in (0, 1)]

    # Per-dst-node z~ biases, rank-64 factorized (pe has numerical rank ~40,
    # so rank 64 is exact to fp32 precision): D = Bfac @ Wfac
    ctl_m1 = np.vstack([np.zeros((1, H), np.float32), ctl[:-1]])   # ctl[n-1]
    ctl_p1 = np.vstack([ctl[1:], np.zeros((1, H), np.float32)])    # ctl[n+1]
    Dfull = {
        "l": ctl_m1 + ctr,
        "r": ctl_p1 + ctr,
        "s": ctl + ctr,
    }
    # p/q/y are linear in x: computed on host directly from the input
    wp = Wl_full @ a                                # [64]
    wq = Wr_full @ a
    Wy = Wl_full @ W_fc                             # [64,3]

    COEF = np.zeros((128, 2), np.float32)
    COEF[:, 0] = s[0:128]
    COEF[:, 1] = s[128:256]
    consts["CONSTS"] = bf(COEF)

    # ---- fp8 DoubleRow operands ----
    def fac128(Dm):
        U, S, Vt = np.linalg.svd(Dm.astype(np.float64), full_matrices=False)
        k = Dm.shape[0]          # rank <= 100, exact
        rs = np.sqrt(S[:k])
        Bf = np.zeros((Dm.shape[0], 128), np.float64)
        Bf[:, :k] = U[:, :k] * rs[None, :]
        Wf = np.zeros((128, Dm.shape[1]), np.float64)
        Wf[:k] = rs[:, None] * Vt[:k]
        return Bf.astype(np.float32), Wf.astype(np.float32)

    BfL, WfL = fac128(Dfull["l"])
    BfR, WfR = fac128(Dfull["r"])
    BfS, WfS = fac128(Dfull["s"])

    def pow2scale(*arrs):
        mx = max(float(np.abs(a).max()) for a in arrs)
        return float(2.0 ** np.floor(np.log2(120.0 / mx)))

    SC_l = pow2scale(S_lr_np[0], S_lr_np[1], WfL)
    SC_r = pow2scale(S_rl_np[0], S_rl_np[1], WfR)
    SC_s = pow2scale(S_s_np[0], S_s_np[1], WfS)

    # stationary pair order matches the X8 K-block pair each stencil reads:
    # l: (x, basisL); r: (basisR, x); s: (basisS, x)
    C8 = np.zeros((128, 12, 128), np.float32)
    for b in (0, 1):
        C8[:, 0 + 2 * b] = S_lr_np[b] * SC_l
        C8[:, 1 + 2 * b] = WfL[:, 128 * b : 128 * b + 128] * SC_l
        C8[:, 4 + 2 * b] = WfR[:, 128 * b : 128 * b + 128] * SC_r
        C8[:, 5 + 2 * b] = S_rl_np[b] * SC_r
        C8[:, 8 + 2 * b] = WfS[:, 128 * b : 128 * b + 128] * SC_s
        C8[:, 9 + 2 * b] = S_s_np[b] * SC_s
    consts["C8D"] = np.ascontiguousarray(C8.astype(NPFP8))

    colpat = np.arange(XP) % 100
    basisL8 = BfL.T[:, colpat].astype(NPFP8)                    # [128, XP]
    basisR8 = BfR.T[:, (np.arange(XP) - 1) % 100].astype(NPFP8)
    basisS8 = BfS.T[:, colpat].astype(NPFP8)

    # per-core inputs
    xr = x.reshape(NCORES, ROWS, IN)
    in_maps = []
    for c in range(NCORES):
        m = dict(consts)
        xcat = np.zeros((128, XP), np.float32)
        xcat[64:128, 0:ROWS] = xr[c].T
        xcat[0:64, 1 : ROWS + 1] = xr[c].T
        xc8 = xcat.astype(NPFP8)
        x8np = np.empty((128, 2, 4, XH), NPFP8)
        for h, base in enumerate((0, XB)):
            x8np[:, h, 0] = basisS8[:, base : base + XH]
            x8np[:, h, 1] = basisR8[:, base : base + XH]
            x8np[:, h, 2] = xc8[:, base : base + XH]
            x8np[:, h, 3] = basisL8[:, base : base + XH]
        m["X8"] = x8np
        in_maps.append(m)

    nc = _get_program()
    res = None
    last_exc = None
    for attempt in range(3):
        try:
            res = run_bass_kernel_spmd(
                nc,
                in_maps,
                core_ids=list(range(NCORES)),
            )
            break
        except Exception as e:  # transient device-unrecoverable on first NEFF run
            last_exc = e
            import time as _time

            _time.sleep(2.0)
    if res is None:
        raise last_exc
    LAST_RESULTS = res

    # ---------------- host tail ----------------
    cp = cl @ a                                               # [100]
    cq = cr @ a
    cy = cl @ W_fc                                            # [100,3]
    n_of_r = np.tile(np.arange(N), BC)                        # [ROWS]

    out_all = np.empty((B, C), np.float32)
    for c in range(NCORES):
        oT = np.asarray(res.results[c]["outsT"], np.float32)  # [3, ROWS]
        t_l, t_r, t_s = oT[0] / SC_l, oT[1] / SC_r, oT[2] / SC_s
        xc = xr[c]                                            # [ROWS, 64]

        Pb = xc @ wp + cp[n_of_r]                             # a.xl per row
        Qb = xc @ wq + cq[n_of_r]                             # a.xr per row
        Y = xc @ Wy + cy[n_of_r]                              # xl @ W_fc per row

        Pb_m1 = np.roll(Pb, 1)                                # P at source row r-1
        Pb_p1 = np.roll(Pb, -1)

        # device t_* are sum_h sign(a_h) relu(z~_h); lrelu = 0.2 z + 0.8 relu
        lg_l = 0.2 * (Pb_m1 + Qb) + 0.8 * t_l
        lg_r = 0.2 * (Pb_p1 + Qb) + 0.8 * t_r
        lg_s = 0.2 * (Pb + Qb) + 0.8 * t_s

        lg_l = np.where(n_of_r == 0, -np.inf, lg_l)
        lg_r = np.where(n_of_r == N - 1, -np.inf, lg_r)

        mx = np.maximum(np.maximum(lg_l, lg_r), lg_s)
        el = np.exp(lg_l - mx)
        er = np.exp(lg_r - mx)
        es = np.exp(lg_s - mx)
        den = el + er + es
        al, ar, asf = el / den, er / den, es / den

        Y_m1 = np.roll(Y, 1, axis=0)
        Y_p1 = np.roll(Y, -1, axis=0)
        msgs = al[:, None] * Y_m1 + ar[:, None] * Y_p1 + asf[:, None] * Y
        pooled = msgs.reshape(BC, N, C).sum(axis=1)
        out_all[c * BC : (c + 1) * BC] = (
            pooled + N * (bias[lw] @ W_fc)[None, :] + b_fc[None, :]
        )
    return out_all
